# revision 1
# baseline (speedup 1.0000x reference)
"""Bass/Tile TRN2 kernel for nn_Attention_12704513261709.

8-way head-parallel attention: each of the 8 NeuronCores computes one head
(dh = 2048) over both batches, plus its partial (row-parallel) output
projection. Host sums the 8 partials.

Shapes (from reference.setup_inputs):
  x: (2, 2048, 256) f32, gamma: (256,), Wq/Wk/Wv: (16384, 256), Wo: (256, 16384)
"""

import numpy as np
import ml_dtypes

B = 2
N_SEQ = 2048
N_TOK = B * N_SEQ  # 4096
D = 256
HEADS = 8
INNER = 16384
DH = INNER // HEADS  # 2048
SCALE = 64 ** (-0.5)
EPS = 1e-5

FT = DH // 128  # 16 f-tiles per head dim
TT = N_SEQ // 128  # 16 key tiles per batch
NCH = N_SEQ // 512  # 4 query chunks of 512 per batch

_CACHE = {}


def _build():
    from concourse import bacc, bass_isa
    import concourse.tile as tile
    import concourse.mybir as mybir
    from concourse.masks import make_identity

    f32 = mybir.dt.float32
    bf16 = mybir.dt.bfloat16
    AF = mybir.ActivationFunctionType
    ALU = mybir.AluOpType

    nc = bacc.Bacc("TRN2", target_bir_lowering=False, debug=False, num_devices=8)

    x_d = nc.dram_tensor("x", [N_TOK, D], f32, kind="ExternalInput").ap()
    wqT_d = nc.dram_tensor("wqT", [D, DH], bf16, kind="ExternalInput").ap()
    wkT_d = nc.dram_tensor("wkT", [D, DH], bf16, kind="ExternalInput").ap()
    wvT_d = nc.dram_tensor("wvT", [D, DH], bf16, kind="ExternalInput").ap()
    woT_d = nc.dram_tensor("woT", [DH, D], bf16, kind="ExternalInput").ap()
    out_d = nc.dram_tensor("outT", [D, N_TOK], f32, kind="ExternalOutput").ap()

    with tile.TileContext(nc) as tc:
        with (
            tc.tile_pool(name="singles", bufs=1) as singles,
            tc.tile_pool(name="ln", bufs=4) as ln_pool,
            tc.tile_pool(name="big", bufs=1) as big,
            tc.tile_pool(name="qt", bufs=1) as qt_pool,
            tc.tile_pool(name="pt", bufs=1) as pt_pool,
            tc.tile_pool(name="ot", bufs=1) as ot_pool,
            tc.tile_pool(name="vstrip", bufs=5) as vs_pool,
            tc.tile_pool(name="stage", bufs=2) as stage_pool,
            tc.tile_pool(name="rsum", bufs=1) as rsum_pool,
            tc.tile_pool(name="dram", bufs=2, space="DRAM") as dram_pool,
            tc.tile_pool(name="psA", bufs=4, space="PSUM") as psA,
            tc.tile_pool(name="psB", bufs=2, space="PSUM") as psB,
            tc.tile_pool(name="psM", bufs=2, space="PSUM") as psM,
        ):
            identity = singles.tile([128, 128], f32)
            make_identity(nc, identity)
            eps_t = singles.tile([128, 1], f32)
            nc.vector.memset(eps_t, EPS)
            warm = singles.tile([128, 1], f32)
            nc.scalar.activation(warm[:], eps_t[:], func=AF.Sqrt, bias=eps_t[:], scale=1.0)
            # dummy matmuls fill the pre-work window (x DMA + LN chain latency)
            # so the HAM clock-gate is already at 8/8 when real matmuls arrive
            dummy_w = singles.tile([128, 128], bf16)
            nc.vector.memset(dummy_w, 0.0)
            dummy_r = singles.tile([128, 256], bf16)
            nc.vector.memset(dummy_r, 0.0)
            for _ in range(32):
                ps = psM.tile([128, 512], f32, tag="m", name="hamwarm")
                nc.tensor.matmul(ps[:, :256], dummy_w[:], dummy_r[:], start=True, stop=True)

            # weights to SBUF
            wqT = [big.tile([128, DH], bf16, tag=f"wq{d_}", name=f"wq{d_}") for d_ in range(2)]
            wkT = [big.tile([128, DH], bf16, tag=f"wk{d_}", name=f"wk{d_}") for d_ in range(2)]
            wvT = [big.tile([128, DH], bf16, tag=f"wv{d_}", name=f"wv{d_}") for d_ in range(2)]
            woT = [big.tile([128, D], bf16, tag=f"wo{fc}", name=f"wo{fc}") for fc in range(FT)]

            xnT = [big.tile([128, N_TOK], bf16, tag=f"xnT{d_}", name=f"xnT{d_}") for d_ in range(2)]

            state = {}

            def ln_chain(i):
                """LayerNorm token tile i (128 tokens): DVE/ACT chain only."""
                x_t = ln_pool.tile([128, D], f32, tag="x", name="x")
                nc.sync.dma_start(x_t[:], x_d[i * 128 : (i + 1) * 128, :])
                stats = ln_pool.tile([128, nc.vector.BN_STATS_DIM], f32, tag="st", name="st")
                nc.vector.bn_stats(stats[:], x_t[:])
                mv = ln_pool.tile([128, nc.vector.BN_AGGR_DIM], f32, tag="mv", name="mv")
                nc.vector.bn_aggr(mv[:], stats[:])
                std = ln_pool.tile([128, 1], f32, tag="std", name="std")
                nc.scalar.activation(
                    std[:], mv[:, 1:2], func=AF.Sqrt, bias=eps_t[:], scale=1.0
                )
                rstd = ln_pool.tile([128, 1], f32, tag="rstd", name="rstd")
                nc.vector.reciprocal(rstd[:], std[:])
                xn_t = ln_pool.tile([128, D], f32, tag="xn", name="xn")
                nc.vector.tensor_scalar(
                    xn_t[:],
                    x_t[:],
                    scalar1=mv[:, 0:1],
                    scalar2=rstd[:],
                    op0=ALU.subtract,
                    op1=ALU.mult,
                )
                state[f"xn{i % 8}"] = xn_t

            def ln_transpose(i):
                xn_t = state[f"xn{i % 8}"]
                for d_ in range(2):
                    ps = psM.tile([128, 512], f32, tag="m", name="m")
                    nc.tensor.transpose(
                        ps[:, :128], xn_t[:, d_ * 128 : (d_ + 1) * 128], identity[:]
                    )
                    nc.any.tensor_copy(xnT[d_][:, i * 128 : (i + 1) * 128], ps[:, :128])

            def ln_tile(i):
                ln_chain(i)
                ln_transpose(i)

            def kt_build_group(bb, nch):
                """K^T columns for one 512-token group of batch bb."""
                base = bb * N_SEQ
                for ft in range(FT):
                    ps = psM.tile([128, 512], f32, tag="m", name="m")
                    for d_ in range(2):
                        nc.tensor.matmul(
                            ps[:],
                            wkT[d_][:, ft * 128 : (ft + 1) * 128],
                            xnT[d_][:, base + nch * 512 : base + (nch + 1) * 512],
                            start=(d_ == 0),
                            stop=(d_ == 1),
                        )
                    nc.any.tensor_copy(
                        state[f"kt{ft}"][:, nch * 512 : (nch + 1) * 512], ps[:]
                    )

            def v_build_tile(bb, t):
                """V rows for key tile t of batch bb -> blocked DRAM scratch."""
                base = bb * N_SEQ
                v_stage = stage_pool.tile([128, DH], bf16, tag="vstage", name="vstage")
                for fch in range(4):
                    ps = psM.tile([128, 512], f32, tag="m", name="m")
                    for d_ in range(2):
                        nc.tensor.matmul(
                            ps[:],
                            xnT[d_][:, base + t * 128 : base + (t + 1) * 128],
                            wvT[d_][:, fch * 512 : (fch + 1) * 512],
                            start=(d_ == 0),
                            stop=(d_ == 1),
                        )
                    nc.any.tensor_copy(v_stage[:, fch * 512 : (fch + 1) * 512], ps[:])
                nc.sync.dma_start(
                    state["v_dram2"][:, :, t, :].rearrange("g p f -> p g f"),
                    v_stage.rearrange("p (g f) -> p g f", g=FT),
                )

            def qt_build(bb, ch):
                cbase = bb * N_SEQ + ch * 512
                state["qt"] = []
                for ft in range(FT):
                    ps = psA.tile([128, 512], f32, tag="st", name="qtps")
                    for d_ in range(2):
                        nc.tensor.matmul(
                            ps[:],
                            wqT[d_][:, ft * 128 : (ft + 1) * 128],
                            xnT[d_][:, cbase : cbase + 512],
                            start=(d_ == 0),
                            stop=(d_ == 1),
                        )
                    qt = qt_pool.tile([128, 512], bf16, tag=f"qt{ft}", name=f"qt{ft}")
                    nc.any.tensor_copy(qt[:], ps[:])
                    state["qt"].append(qt)

            def phase_a(bb, ch, extras=()):
                """S^T + exp per key tile; prefetch V strips. Rowsum is done
                entirely off the PE: DVE reduce over key tiles, GpSimd
                partition all-reduce (which also broadcasts), DVE reciprocal."""
                QT = state["qt"]
                KT = [state[f"kt{ft}"] for ft in range(FT)]
                pt_big = pt_pool.tile([128, TT * 512], bf16, tag="pt", name="pt")
                state["pt_big"] = pt_big
                state["pt"] = [
                    pt_big[:, t * 512 : (t + 1) * 512] for t in range(TT)
                ]
                state["strips"] = []
                for t in range(TT):
                    strip = vs_pool.tile([128, TT, 128], bf16, tag="vstrip", name="vstrip")
                    nc.gpsimd.dma_start(strip[:], state["v_dram"][t])
                    state["strips"].append(strip)
                    st_ps = psA.tile([128, 512], f32, tag="st", name="st")
                    for ft in range(FT):
                        nc.tensor.matmul(
                            st_ps[:],
                            KT[ft][:, t * 128 : (t + 1) * 128],
                            QT[ft][:],
                            start=(ft == 0),
                            stop=(ft == FT - 1),
                        )
                    nc.scalar.activation(state["pt"][t], st_ps[:], func=AF.Exp)
                    if t < len(extras):
                        extras[t]()
                for j in range(TT, len(extras)):
                    extras[j]()

            def phase_rsum():
                rsum_p = rsum_pool.tile([128, 512], f32, tag="rsum_p", name="rsum_p")
                nc.vector.tensor_reduce(
                    rsum_p[:],
                    state["pt_big"].rearrange("p (t i) -> p i t", t=TT),
                    axis=mybir.AxisListType.X,
                    op=ALU.add,
                )
                rsum_all = rsum_pool.tile([128, 512], f32, tag="rsum_a", name="rsum_a")
                nc.gpsimd.partition_all_reduce(
                    rsum_all[:], rsum_p[:], channels=128,
                    reduce_op=bass_isa.ReduceOp.add,
                )
                rbc = rsum_pool.tile([128, 512], f32, tag="rbc", name="rbc")
                nc.vector.reciprocal(rbc[:], rsum_all[:])
                state["rbc"] = rbc

            def phase_b(extras=(), fc_start=0, fc_stop=FT):
                PT = state["pt"]
                if fc_start == 0:
                    state["ot"] = []
                for fc in range(fc_start, fc_stop):
                    if fc % 3 == 2 and fc // 3 < len(extras):
                        extras[fc // 3]()
                    strip = state["strips"][fc]
                    ot_ps = psB.tile([128, 512], f32, tag="ot", name="ot")
                    for t in range(TT):
                        nc.tensor.matmul(
                            ot_ps[:],
                            strip[:, t, :],
                            PT[t][:],
                            start=(t == 0),
                            stop=(t == TT - 1),
                        )
                    ot = ot_pool.tile([128, 512], bf16, tag=f"ot{fc}", name=f"ot{fc}")
                    nc.any.tensor_copy(ot[:], ot_ps[:])
                    state["ot"].append(ot)
                for j in range(FT // 3, len(extras)):
                    extras[j]()

            def phase_c(bb, ch):
                cbase = bb * N_SEQ + ch * 512
                OT = state["ot"]
                for dm in range(2):
                    op_ps = psM.tile([128, 512], f32, tag="m", name="m")
                    for fc in range(FT):
                        nc.tensor.matmul(
                            op_ps[:],
                            woT[fc][:, dm * 128 : (dm + 1) * 128],
                            OT[fc][:],
                            start=(fc == 0),
                            stop=(fc == FT - 1),
                        )
                    op_sb = stage_pool.tile([128, 512], f32, tag="opsb", name="opsb")
                    nc.vector.tensor_tensor(
                        op_sb[:], op_ps[:], state["rbc"][:], ALU.mult
                    )
                    nc.sync.dma_start(
                        out_d[dm * 128 : (dm + 1) * 128, cbase : cbase + 512],
                        op_sb[:],
                    )

            def kt_alloc(bb):
                for ft in range(FT):
                    state[f"kt{ft}"] = big.tile(
                        [128, N_SEQ], bf16, tag=f"kt{ft}", name=f"kt{ft}"
                    )

            def v_alloc(bb):
                state["v_dram2"] = dram_pool.tile(
                    [FT, 128, TT, 128], bf16, tag="vscratch", name="vscratch"
                )

            def v_promote():
                state["v_dram"] = state["v_dram2"]

            # ---- prologue: batch-0 LN interleaved with batch-0 K/V builds,
            # V/KT lagging one tile so transpose->copy latency stays hidden ----
            kt_alloc(0)
            v_alloc(0)
            v_promote()
            for i in range(3):
                ln_chain(i)  # x loads go first on the sync queue
            nc.sync.dma_start(wvT[0][:], wvT_d[0:128, :])
            nc.gpsimd.dma_start(wvT[1][:], wvT_d[128:256, :])
            for d_ in range(2):
                sl = slice(d_ * 128, (d_ + 1) * 128)
                nc.gpsimd.dma_start(wkT[d_][:], wkT_d[sl, :])
                nc.gpsimd.dma_start(wqT[d_][:], wqT_d[sl, :])
            for fc in range(FT):
                nc.gpsimd.dma_start(woT[fc][:], woT_d[fc * 128 : (fc + 1) * 128, :])
            for i in range(TT):
                if i + 3 < TT:
                    ln_chain(i + 3)
                ln_transpose(i)
                if i > 0:
                    v_build_tile(0, i - 1)
                if i % 4 == 1 and i > 4:
                    kt_build_group(0, i // 4 - 1)
            v_build_tile(0, TT - 1)
            for g in (2, 3):
                kt_build_group(0, g)
            qt_build(0, 0)

            # ---- main loop over 8 chunks, with batch-1 prep woven into the
            # phase A/B matmul streams of batch-0 chunks ----
            def mk(f, *a):
                return lambda: f(*a)

            a_extras = {
                # batch-1 layernorm spread over chunks 0..2 (one per t-group)
                (0, 0): [mk(ln_tile, i) for i in range(TT, TT + 6)],
                # chunk 1: rest of LN + start batch-1 V (needs ln 16..21)
                (0, 1): [mk(ln_tile, i) for i in range(TT + 6, TT + 12)]
                + [mk(v_build_tile, 1, t) for t in range(0, 4)],
                (0, 2): [mk(ln_tile, i) for i in range(TT + 12, TT + 16)]
                + [mk(v_build_tile, 1, t) for t in range(4, 10)],
                (0, 3): [mk(v_build_tile, 1, t) for t in range(10, TT)],
            }
            b_extras = {
                # batch-1 K^T + next QT woven between B fc-groups of (0,3);
                # the copies only flow after A(0,3) reads finish, which holds
                (0, 3): [mk(kt_build_group, 1, g) for g in range(4)]
                + [mk(qt_build, 1, 0)],
            }
            for bb, ch in [(b_, c_) for b_ in range(B) for c_ in range(NCH)]:
                if (bb, ch) == (0, 1):
                    v_alloc(1)  # batch-1 scratch; strips still read batch-0's
                if (bb, ch) == (1, 0):
                    v_promote()  # batch-1 V scratch becomes current
                phase_a(bb, ch, a_extras.get((bb, ch), ()))
                if ch < NCH - 1:
                    qt_build(bb, ch + 1)
                phase_rsum()
                if (bb, ch) == (0, 3):
                    kt_alloc(1)
                phase_b(b_extras.get((bb, ch), ()))
                phase_c(bb, ch)

    nc.compile()
    return nc


def get_nc():
    if "nc" not in _CACHE:
        _CACHE["nc"] = _build()
    return _CACHE["nc"]


def make_in_maps(x, gamma, Wq, Wk, Wv, Wo):
    bf = ml_dtypes.bfloat16
    gp = (1.0 + gamma.astype(np.float64))[None, :]
    x_flat = np.ascontiguousarray(x.reshape(N_TOK, D).astype(np.float32))
    in_maps = []
    for h in range(HEADS):
        sl = slice(h * DH, (h + 1) * DH)
        wq = (Wq[sl].astype(np.float64) * gp * SCALE).T.astype(bf)
        wk = (Wk[sl].astype(np.float64) * gp).T.astype(bf)
        wv = (Wv[sl].astype(np.float64) * gp).T.astype(bf)
        wo = Wo[:, sl].T.astype(bf)
        in_maps.append(
            {
                "x": x_flat,
                "wqT": np.ascontiguousarray(wq),
                "wkT": np.ascontiguousarray(wk),
                "wvT": np.ascontiguousarray(wv),
                "woT": np.ascontiguousarray(wo),
            }
        )
    return in_maps


def kernel(x, gamma, Wq, Wk, Wv, Wo):
    from concourse import bass_utils

    x, gamma, Wq, Wk, Wv, Wo = (
        np.asarray(a) for a in (x, gamma, Wq, Wk, Wv, Wo)
    )
    nc = get_nc()
    in_maps = make_in_maps(x, gamma, Wq, Wk, Wv, Wo)
    res = bass_utils.run_bass_kernel_spmd(
        nc, in_maps, core_ids=list(range(HEADS))
    )
    acc = np.zeros((D, N_TOK), np.float32)
    for h in range(HEADS):
        acc += res.results[h]["outT"]
    return np.ascontiguousarray(acc.T).reshape(B, N_SEQ, D).astype(np.float32)



# revision 3
# speedup vs baseline: 5.7528x; 5.7528x over previous
"""Bass/Tile TRN2 kernel for nn_Attention_12704513261709.

Algebraic reduction: per head h (dh=2048 > d=256), fold the projections into
two 256x256 matrices on the host:
    M'_h = diag(1+gamma) . (scale . Wq_h^T Wk_h) . diag(1+gamma)
    P'_h = (Wo[:, h] . Wv_h) . diag(1+gamma)
Then with xh = plain layernorm(x) (no gamma):
    S_h   = xh M'_h xh^T                (logits, per batch)
    out   = sum_h softmax(S_h) xh P'_h^T
This cuts per-core PE work ~9x vs materializing q/k/v in dh=2048.

8-way head-parallel: core h computes head h for both batches; host sums the
8 partial outputs. Per-core device work: LN -> xnT (PE transpose) ->
Y^T = M'^T xn^T, Z = xn P'^T (+ ones column for the softmax row-sum) ->
per 512-query chunk: S^T tiles -> exp -> A^T @ [Z|1] accumulated over key
tiles (rowsum rides along as column 256) -> scale by 1/rowsum -> DMA out.

Shapes: x (2,2048,256) f32, gamma (256,), Wq/Wk/Wv (16384,256), Wo (256,16384).
"""

import numpy as np
import ml_dtypes

B = 2
N_SEQ = 2048
N_TOK = B * N_SEQ  # 4096
D = 256
HEADS = 8
DH = 2048  # per-head dim of the original module (16384/8)
SCALE = 64 ** (-0.5)
EPS = 1e-5

TT = N_SEQ // 128  # 16 key tiles per batch
NCH = N_SEQ // 512  # 4 query chunks of 512 per batch

_CACHE = {}


def _build():
    from concourse import bacc
    import concourse.tile as tile
    import concourse.mybir as mybir
    from concourse.masks import make_identity

    f32 = mybir.dt.float32
    bf16 = mybir.dt.bfloat16
    AF = mybir.ActivationFunctionType
    ALU = mybir.AluOpType

    nc = bacc.Bacc("TRN2", target_bir_lowering=False, debug=False, num_devices=8)

    x_d = nc.dram_tensor("x", [N_TOK, D], f32, kind="ExternalInput").ap()
    m_d = nc.dram_tensor("m", [D, D], bf16, kind="ExternalInput").ap()
    pT_d = nc.dram_tensor("pT", [D, D], bf16, kind="ExternalInput").ap()
    o_d = nc.dram_tensor("o_part", [N_TOK, D], f32, kind="ExternalOutput").ap()

    with tile.TileContext(nc) as tc:
        with (
            tc.tile_pool(name="singles", bufs=1) as singles,
            tc.tile_pool(name="ln", bufs=6) as ln_pool,
            tc.tile_pool(name="stage", bufs=4) as stage_pool,
            tc.tile_pool(name="ptp", bufs=1) as pt_pool,
            tc.tile_pool(name="psS", bufs=2, space="PSUM") as psS,
            tc.tile_pool(name="psO", bufs=4, space="PSUM") as psO,
            tc.tile_pool(name="psT", bufs=2, space="PSUM") as psT,
        ):
            identity = singles.tile([128, 128], bf16)
            make_identity(nc, identity)
            eps_t = singles.tile([128, 1], f32)
            nc.vector.memset(eps_t, EPS)

            # dummy matmuls keep the PE clock-gate warm during the prologue
            dummy_w = singles.tile([128, 128], bf16)
            nc.vector.memset(dummy_w, 0.0)
            dummy_r = singles.tile([128, 256], bf16)
            nc.vector.memset(dummy_r, 0.0)

            def dummy_mm():
                ps = psS.tile([128, 512], f32, tag="s", name="warm")
                nc.tensor.matmul(ps[:, :256], dummy_w[:], dummy_r[:], start=True, stop=True)

            for _ in range(16):
                dummy_mm()

            # small folded weights
            m_sb = [singles.tile([128, D], bf16, name=f"msb{i}") for i in range(2)]
            pT_sb = [singles.tile([128, D], bf16, name=f"ptsb{i}") for i in range(2)]

            xnT = [
                [singles.tile([128, N_SEQ], bf16, name=f"xnT{bb}{d_}") for d_ in range(2)]
                for bb in range(B)
            ]
            yT = [
                [singles.tile([128, N_SEQ], bf16, name=f"yT{bb}{d_}") for d_ in range(2)]
                for bb in range(B)
            ]
            # Z' = [xh P'^T | 1]: per key tile, 257 cols (col 256 = ones)
            zp = [
                [singles.tile([128, 257], bf16, name=f"zp{bb}{t}") for t in range(TT)]
                for bb in range(B)
            ]
            for bb in range(B):
                for t in range(TT):
                    nc.gpsimd.memset(zp[bb][t][:, 256:257], 1.0)

            state = {}

            def ln_chain(bb, i):
                """LayerNorm token tile (bb, i): DMA + DVE/ACT chain -> bf16 xn."""
                gi = bb * TT + i
                x_t = ln_pool.tile([128, D], f32, tag="x", name="x")
                nc.sync.dma_start(x_t[:], x_d[gi * 128 : (gi + 1) * 128, :])
                stats = ln_pool.tile([128, nc.vector.BN_STATS_DIM], f32, tag="st", name="st")
                nc.vector.bn_stats(stats[:], x_t[:])
                mv = ln_pool.tile([128, nc.vector.BN_AGGR_DIM], f32, tag="mv", name="mv")
                nc.vector.bn_aggr(mv[:], stats[:])
                std = ln_pool.tile([128, 1], f32, tag="sd", name="sd")
                nc.scalar.activation(std[:], mv[:, 1:2], func=AF.Sqrt, bias=eps_t[:], scale=1.0)
                rstd = ln_pool.tile([128, 1], f32, tag="rs", name="rs")
                nc.vector.reciprocal(rstd[:], std[:])
                xn_t = ln_pool.tile([128, D], bf16, tag="xn", name="xn")
                nc.vector.tensor_scalar(
                    xn_t[:],
                    x_t[:],
                    scalar1=mv[:, 0:1],
                    scalar2=rstd[:],
                    op0=ALU.subtract,
                    op1=ALU.mult,
                )
                state[gi % 8] = xn_t

            def ln_transpose(bb, i):
                gi = bb * TT + i
                xn_t = state[gi % 8]
                tp = psT.tile([128, 256], bf16, tag="tp", name="tp")
                for d_ in range(2):
                    nc.tensor.transpose(
                        tp[:, d_ * 128 : (d_ + 1) * 128],
                        xn_t[:, d_ * 128 : (d_ + 1) * 128],
                        identity[:],
                    )
                for d_ in range(2):
                    nc.any.tensor_copy(
                        xnT[bb][d_][:, i * 128 : (i + 1) * 128],
                        tp[:, d_ * 128 : (d_ + 1) * 128],
                    )

            def ln_tile(bb, i):
                ln_chain(bb, i)
                ln_transpose(bb, i)

            def z_build(bb, t):
                """Z rows for key tile t: [128 tok, 256] = xn_tile @ P'^T."""
                ps = psS.tile([128, 512], f32, tag="s", name="zps")
                for d_ in range(2):
                    nc.tensor.matmul(
                        ps[:, :256],
                        xnT[bb][d_][:, t * 128 : (t + 1) * 128],
                        pT_sb[d_][:],
                        start=(d_ == 0),
                        stop=(d_ == 1),
                    )
                nc.any.tensor_copy(zp[bb][t][:, :256], ps[:, :256])

            def yt_build(bb, c, dm):
                """Y^T[dm-half, 512-query chunk c] = M'^T xn^T."""
                ps = psS.tile([128, 512], f32, tag="s", name="yps")
                for d_ in range(2):
                    nc.tensor.matmul(
                        ps[:],
                        m_sb[d_][:, dm * 128 : (dm + 1) * 128],
                        xnT[bb][d_][:, c * 512 : (c + 1) * 512],
                        start=(d_ == 0),
                        stop=(d_ == 1),
                    )
                nc.any.tensor_copy(yT[bb][dm][:, c * 512 : (c + 1) * 512], ps[:])

            def av_step(bb, t, pt_t, av_ps):
                for qt in range(4):
                    nc.tensor.matmul(
                        av_ps[qt][:, :257],
                        pt_t[:, qt * 128 : (qt + 1) * 128],
                        zp[bb][t][:, :257],
                        start=(t == 0),
                        stop=(t == TT - 1),
                    )

            def chunk(bb, ch, extras=()):
                """One 512-query chunk: S^T tiles -> exp -> AV (interleaved),
                then 1/rowsum scale + output DMA."""
                cq = ch * 512
                gbase = bb * N_SEQ + cq
                av_ps = [
                    psO.tile([128, 512], f32, tag="o", name=f"av{qt}") for qt in range(4)
                ]
                pts = []
                ei = [0]

                def run_extra():
                    if ei[0] < len(extras):
                        extras[ei[0]]()
                        ei[0] += 1

                for t in range(TT):
                    sps = psS.tile([128, 512], f32, tag="s", name="sps")
                    for d_ in range(2):
                        nc.tensor.matmul(
                            sps[:],
                            xnT[bb][d_][:, t * 128 : (t + 1) * 128],
                            yT[bb][d_][:, cq : cq + 512],
                            start=(d_ == 0),
                            stop=(d_ == 1),
                        )
                    pt_t = pt_pool.tile([128, 512], bf16, tag=f"pt{t}", name=f"pt{t}")
                    nc.scalar.activation(pt_t[:], sps[:], func=AF.Exp)
                    pts.append(pt_t)
                    if t > 0:
                        av_step(bb, t - 1, pts[t - 1], av_ps)
                    run_extra()
                av_step(bb, TT - 1, pts[TT - 1], av_ps)
                while ei[0] < len(extras):
                    run_extra()
                for qt in range(4):
                    rcp = stage_pool.tile([128, 1], f32, tag="rcp", name="rcp")
                    nc.vector.reciprocal(rcp[:], av_ps[qt][:, 256:257])
                    ob = stage_pool.tile([128, 256], f32, tag="ob", name="ob")
                    nc.vector.tensor_scalar(
                        ob[:], av_ps[qt][:, :256], scalar1=rcp[:], scalar2=None,
                        op0=ALU.mult,
                    )
                    nc.gpsimd.dma_start(
                        o_d[gbase + qt * 128 : gbase + (qt + 1) * 128, :], ob[:]
                    )

            # ---- prologue: batch-0 LN/transpose pipelined with Z/Y^T builds ----
            for i in range(4):
                ln_chain(0, i)
            for i in range(2):
                nc.gpsimd.dma_start(m_sb[i][:], m_d[i * 128 : (i + 1) * 128, :])
                nc.gpsimd.dma_start(pT_sb[i][:], pT_d[i * 128 : (i + 1) * 128, :])
            for t in range(TT):
                if t + 4 < TT:
                    ln_chain(0, t + 4)
                ln_transpose(0, t)
                dummy_mm()
                z_build(0, t)
                if t % 4 == 3:
                    yt_build(0, t // 4, 0)
                    yt_build(0, t // 4, 1)

            # ---- main loop; batch-1 prep woven into batch-0 chunks ----
            def mk(f, *a):
                return lambda: f(*a)

            ex01 = []
            for j in range(8):
                ex01.append(mk(ln_tile, 1, 8 + j))
                ex01.append(mk(z_build, 1, j))
            ex02 = [mk(z_build, 1, 8 + j) for j in range(8)] + [
                mk(yt_build, 1, c, dm) for c in range(NCH) for dm in range(2)
            ]
            a_extras = {
                (0, 0): [mk(ln_tile, 1, j) for j in range(8)],
                (0, 1): ex01,
                (0, 2): ex02,
            }
            for bb in range(B):
                for ch in range(NCH):
                    chunk(bb, ch, a_extras.get((bb, ch), ()))

    nc.compile()
    return nc


def get_nc():
    if "nc" not in _CACHE:
        _CACHE["nc"] = _build()
    return _CACHE["nc"]


def make_in_maps(x, gamma, Wq, Wk, Wv, Wo):
    bf = ml_dtypes.bfloat16
    g = 1.0 + gamma.astype(np.float64)
    x_flat = np.ascontiguousarray(x.reshape(N_TOK, D).astype(np.float32))
    Wq64, Wk64, Wv64, Wo64 = (a.astype(np.float64) for a in (Wq, Wk, Wv, Wo))
    in_maps = []
    for h in range(HEADS):
        sl = slice(h * DH, (h + 1) * DH)
        M = SCALE * (Wq64[sl].T @ Wk64[sl]) * g[:, None] * g[None, :]
        PT = ((Wo64[:, sl] @ Wv64[sl]) * g[None, :]).T
        in_maps.append(
            {
                "x": x_flat,
                "m": np.ascontiguousarray(M.astype(bf)),
                "pT": np.ascontiguousarray(PT.astype(bf)),
            }
        )
    return in_maps


def gather(results):
    acc = np.zeros((N_TOK, D), np.float32)
    for h in range(HEADS):
        acc += results[h]["o_part"]
    return acc.reshape(B, N_SEQ, D)


def kernel(x, gamma, Wq, Wk, Wv, Wo):
    from concourse import bass_utils

    x, gamma, Wq, Wk, Wv, Wo = (
        np.asarray(a) for a in (x, gamma, Wq, Wk, Wv, Wo)
    )
    nc = get_nc()
    in_maps = make_in_maps(x, gamma, Wq, Wk, Wv, Wo)
    res = bass_utils.run_bass_kernel_spmd(
        nc, in_maps, core_ids=list(range(HEADS))
    )
    return gather(res.results).astype(np.float32)
